# revision 4
# baseline (speedup 1.0000x reference)
"""GCN (2-layer, mean/add/min/max aggregation) Trainium2 Bass kernel, 8 NeuronCores.

v3: eliminates all table-build dma_gather descriptors (Q7 SWDGE descriptor
generation at ~6-8ns/desc is the machine bottleneck for indexed access).

Per core (5000 dest nodes, 2 dest-phases x 2560 padded lanes = 40 blocks):
- Token space NG=40960 split asymmetrically for int16 gather indices:
  part1 = 32768 tokens (cols 0..4095 of each core), part2 = 8192 tokens
  (cols 4096..5119 = each core's 1024 lowest-OUT-degree nodes, so only ~13%
  of edge sources hit part2).
- SBUF-resident full g-table [128, 320, 128] bf16, token-major. Layer 1:
  built by PE matmul from a streamed feature-major xs (= dinv*x) input.
  Layer 2: cores compute g2 = dinv*(h1 @ W1.T) for own nodes (A-phase), two
  phase-half AllGathers move it, table bulk-streamed back with large HWDGE
  descriptors. Zero 256B random table gathers.
- Edge messages: per (layer, phase, part) SBUF->SBUF dma_gather from the part
  view (relative idx < 32768), lane-major uniform slots per 128-dest block,
  tensor_reduce add/min/max + pad correction (pad = slot0 dup). Dests sorted
  by part1-count (~total degree) within col ranges to minimize slot padding.
- The appended self-loop message is NOT gathered: g_self columns (one matmul
  on a contiguous slice) merge into add/min/max afterward. Lanes with no
  sources in a part are neutralized via +-1e30 mask slabs; their partial sum
  self-cancels through the standard pad correction.
"""
import sys

sys.path.insert(0, "/opt/trn_rl_repo")

import numpy as np
import ml_dtypes
from contextlib import ExitStack

import concourse.bacc as bacc
import concourse.tile as tile
import concourse.mybir as mybir
from concourse import bass_utils

N = 40000
E = 640000
D = 128
NCLS = 40
CORES = 8
NPC = N // CORES            # 5000 nodes/core
PHASES = 2
BPP = 20                    # blocks per phase
LPP = BPP * 128             # 2560 lanes/phase
NPADC = PHASES * LPP        # 5120 padded nodes/core
NG = CORES * NPADC          # 40960 tokens
P1C = 4096                  # part1 cols per core
P2C = NPADC - P1C           # 1024 part2 cols per core
NP1 = CORES * P1C           # 32768 part1 tokens
NP2 = CORES * P2C           # 8192
RP1 = NP1 // 128            # 256 part1 ranks
RP2 = NP2 // 128            # 64
MSG_COLS = 4480             # max slots per gather call
BF16 = ml_dtypes.bfloat16
BIG = np.float32(1e30)


def _wrap_idx(idx):
    """int16 -> [128, n/16] wrapped (i -> [i%16, i//16]) and replicated x8."""
    idx = np.asarray(idx, dtype=np.int16)
    n = len(idx)
    assert n % 16 == 0
    cols = n // 16
    base = np.zeros((16, cols), dtype=np.int16)
    base[np.arange(n) % 16, np.arange(n) // 16] = idx
    return np.tile(base, (8, 1))


def _host_prep(x, edge_index):
    row_e = np.asarray(edge_index[0], dtype=np.int64)
    col_e = np.asarray(edge_index[1], dtype=np.int64)
    deg_e = np.bincount(col_e, minlength=N)
    out_e = np.bincount(row_e, minlength=N)
    deg = (deg_e + 1).astype(np.float64)
    dinv = deg ** -0.5

    # ---- part2 selection (per-core 1024 lowest out-degree) + counts
    part2 = np.zeros(N, dtype=bool)
    for c in range(CORES):
        sel = np.argsort(out_e[c * NPC:(c + 1) * NPC], kind="stable")[:P2C]
        part2[c * NPC + sel] = True
    cnt2 = np.bincount(col_e[part2[row_e]], minlength=N)
    cnt1 = deg_e - cnt2

    # ---- column assignment: part1 cols 0..4095 / part2 cols 4096..5119,
    # sorted by (part1 cnt desc, part2 cnt desc) within each range
    col_of = np.zeros(N, dtype=np.int64)
    lanes_of = {}           # c -> [5120] node id or -1
    for c in range(CORES):
        nid0 = c * NPC
        loc2 = nid0 + np.nonzero(part2[nid0:nid0 + NPC])[0]
        loc1 = nid0 + np.nonzero(~part2[nid0:nid0 + NPC])[0]
        p1s = loc1[np.lexsort((-cnt2[loc1], -cnt1[loc1]))]
        p2s = loc2[np.lexsort((-cnt2[loc2], -cnt1[loc2]))]
        lanes = np.full(NPADC, -1, dtype=np.int64)
        lanes[:len(p1s)] = p1s
        lanes[P1C:P1C + len(p2s)] = p2s
        col_of[p1s] = np.arange(len(p1s))
        col_of[p2s] = P1C + np.arange(len(p2s))
        lanes_of[c] = lanes
    core_of = np.arange(N) // NPC
    # token id: part1: c*4096+col (< 32768); part2: 32768 + c*1024 + (col-4096)
    tok_of = np.where(~part2, core_of * P1C + col_of,
                      NP1 + core_of * P2C + (col_of - P1C))

    # ---- uniform slot counts, global over cores.
    # part1: 128-lane blocks; part2: 64-lane sub-blocks (less padding), with
    # per-128-block even total so gather calls stay 128-multiple.
    NB = PHASES * BPP
    S1 = np.ones(NB, dtype=np.int64)
    S2 = np.ones(2 * NB, dtype=np.int64)
    for c in range(CORES):
        lanes = lanes_of[c]
        for b in range(NB):
            ls = lanes[b * 128:(b + 1) * 128]
            ls = ls[ls >= 0]
            if len(ls):
                S1[b] = max(S1[b], cnt1[ls].max())
        for u in range(2 * NB):
            ls = lanes[u * 64:(u + 1) * 64]
            ls = ls[ls >= 0]
            if len(ls):
                S2[u] = max(S2[u], cnt2[ls].max())
    for b in range(NB):
        if (S2[2 * b] + S2[2 * b + 1]) % 2:
            S2[2 * b + 1] += 1
    P1 = np.zeros(NB + 1, dtype=np.int64)
    P2 = np.zeros(2 * NB + 1, dtype=np.int64)
    P1[1:] = np.cumsum(128 * S1)
    P2[1:] = np.cumsum(64 * S2)
    LEN1 = [int(P1[(H + 1) * BPP] - P1[H * BPP]) for H in range(PHASES)]
    LEN2 = [int(P2[2 * (H + 1) * BPP] - P2[2 * H * BPP])
            for H in range(PHASES)]

    def chunk_units(S, H, W, upb):
        # chunk whole 128-blocks (upb units per block) into gather calls
        chunks, cur, cols = [], [], 0
        for b in range(H * BPP, (H + 1) * BPP):
            us = list(range(b * upb, (b + 1) * upb))
            w = sum(W * int(S[u]) for u in us)
            if cur and cols + w > MSG_COLS:
                chunks.append(cur)
                cur, cols = [], 0
            cur.extend(us)
            cols += w
        if cur:
            chunks.append(cur)
        return chunks
    chunks1 = [chunk_units(S1, H, 128, 1) for H in range(PHASES)]
    chunks2 = [chunk_units(S2, H, 64, 2) for H in range(PHASES)]

    # ---- per-dest source lists sorted by (dest, part): part1 prefix
    key = col_e * 2 + part2[row_e]
    eorder = np.argsort(key, kind="stable")
    srcs_tok = tok_of[row_e[eorder]]
    offk = np.zeros(2 * N + 1, dtype=np.int64)
    offk[1:] = np.cumsum(np.bincount(key, minlength=2 * N))

    # ---- per-core arrays
    per_core = []
    for c in range(CORES):
        lanes = lanes_of[c]
        ed1 = [np.zeros(LEN1[H], dtype=np.int64) for H in range(PHASES)]
        ed2 = [np.zeros(LEN2[H], dtype=np.int64) for H in range(PHASES)]
        npad1 = np.zeros(NPADC, dtype=np.float32)
        npad2 = np.zeros(NPADC, dtype=np.float32)
        mask1 = np.zeros(NPADC, dtype=np.float32)
        mask2 = np.zeros(NPADC, dtype=np.float32)
        dinv_l = np.zeros(NPADC, dtype=np.float32)
        dsc = np.zeros((128, NB), dtype=np.float32)
        gl = np.zeros(NPADC, dtype=np.int64)
        real = np.zeros(NPADC, dtype=bool)
        units = [(1, b, b * 128, 128, int(S1[b]),
                  int(P1[b] - P1[(b // BPP) * BPP])) for b in range(NB)]
        units += [(2, u, u * 64, 64, int(S2[u]),
                   int(P2[u] - P2[2 * (u // (2 * BPP)) * BPP]))
                  for u in range(2 * NB)]
        for P, b, col0, W, S, Poff in units:
            H = col0 // LPP
            ls = lanes[col0:col0 + W]
            if True:
                ed, npad, mask, base_rel = (
                    (ed1[H], npad1, mask1, 0) if P == 1 else
                    (ed2[H], npad2, mask2, NP1))
                M = np.zeros((W, S), dtype=np.int64)
                for l in range(W):
                    col = col0 + l
                    d = ls[l]
                    if d < 0:
                        # pad lane: own (part1) pad token exists at this col
                        if P == 1:
                            M[l, :] = c * P1C + col
                        else:
                            M[l, :] = 0
                            mask[col] = BIG
                        npad[col] = S
                        continue
                    o0 = offk[2 * d + (P - 1)]
                    o1 = offk[2 * d + P]
                    k = o1 - o0
                    if k:
                        t = srcs_tok[o0:o1] - base_rel
                        M[l, :k] = t
                        M[l, k:] = t[0]
                    else:
                        selfp2 = part2[d]
                        if (P == 2) == selfp2:
                            M[l, :] = tok_of[d] - base_rel   # self fallback
                        else:
                            M[l, :] = 0
                            mask[col] = BIG
                    npad[col] = S - k
                ed[Poff:Poff + W * S] = M.reshape(-1)
        for b in range(NB):
            ls = lanes[b * 128:(b + 1) * 128]
            for l in range(128):
                col = b * 128 + l
                d = ls[l]
                if d >= 0:
                    dinv_l[col] = dinv[d]
                    dsc[l, b] = dinv[d]
                    gl[col] = d
                    real[col] = True

        def slab(a, H):
            return np.broadcast_to(a[H * LPP:(H + 1) * LPP],
                                   (128, LPP)).astype(BF16).copy()
        pcd = dict(
            eidx_p1_0=_wrap_idx(ed1[0]), eidx_p1_1=_wrap_idx(ed1[1]),
            eidx_p2_0=_wrap_idx(ed2[0]), eidx_p2_1=_wrap_idx(ed2[1]),
            dsc=dsc, gl=gl, real=real,
        )
        for H in range(PHASES):
            pcd[f"npad1_{H}"] = slab(npad1, H)
            pcd[f"npad2_{H}"] = slab(npad2, H)
            pcd[f"mask1_{H}"] = slab(mask1, H)
            pcd[f"mask2_{H}"] = slab(mask2, H)
            pcd[f"dinvb_{H}"] = slab(dinv_l, H)
        per_core.append(pcd)

    # ---- xs feature-major, token-ordered, dinv-prescaled
    xs = dinv[:, None] * np.asarray(x, dtype=np.float64)
    xs_fm = np.zeros((D, NG), dtype=np.float32)
    xs_fm[:, tok_of] = xs.T.astype(np.float32)

    meta = dict(S1=S1, S2=S2, P1=P1, P2=P2, LEN1=LEN1, LEN2=LEN2,
                chunks1=chunks1, chunks2=chunks2)
    return per_core, xs_fm.astype(BF16), meta


def _build_program(meta):
    S1, S2, P1, P2 = meta["S1"], meta["S2"], meta["P1"], meta["P2"]
    LEN1, LEN2 = meta["LEN1"], meta["LEN2"]
    chunks1, chunks2 = meta["chunks1"], meta["chunks2"]
    f32, bf16, i16 = mybir.dt.float32, mybir.dt.bfloat16, mybir.dt.int16
    AX = mybir.AxisListType.X
    OP = mybir.AluOpType
    AF = mybir.ActivationFunctionType

    nc = bacc.Bacc("TRN2", target_bir_lowering=False, debug=False,
                   num_devices=CORES)
    t_xs = nc.dram_tensor("xs_fm", [128, NG], bf16, kind="ExternalInput")
    t_w = [nc.dram_tensor(f"W{l}T", [128, 128], bf16, kind="ExternalInput")
           for l in range(2)]
    t_c = [nc.dram_tensor(f"C{l}T", [4, 128, 128], bf16, kind="ExternalInput")
           for l in range(2)]
    t_b = [nc.dram_tensor(f"b{l}", [128, 1], f32, kind="ExternalInput")
           for l in range(2)]
    t_wout = nc.dram_tensor("WoutT", [128, NCLS], bf16, kind="ExternalInput")
    t_bout = nc.dram_tensor("boutb", [128, NCLS], f32, kind="ExternalInput")
    t_dsc = nc.dram_tensor("dsc", [128, PHASES * BPP], f32,
                           kind="ExternalInput")
    t_xs_self = [nc.dram_tensor(f"xs_self{H}", [128, LPP], bf16,
                                kind="ExternalInput") for H in range(PHASES)]
    t_eidx = {}
    for H in range(PHASES):
        t_eidx[1, H] = nc.dram_tensor(
            f"eidx_p1_{H}", [128, LEN1[H] // 16], i16, kind="ExternalInput")
        t_eidx[2, H] = nc.dram_tensor(
            f"eidx_p2_{H}", [128, LEN2[H] // 16], i16, kind="ExternalInput")
    t_slab = {}
    for name in ("npad1", "npad2", "mask1", "mask2", "dinvb"):
        for H in range(PHASES):
            t_slab[name, H] = nc.dram_tensor(
                f"{name}_{H}", [128, LPP], bf16, kind="ExternalInput")
    t_out = nc.dram_tensor("out", [NPADC, NCLS], f32, kind="ExternalOutput")
    t_gsh = [nc.dram_tensor(f"gsh{H}", [128, BPP, 128], bf16, kind="Internal")
             for H in range(PHASES)]
    t_gfull = [nc.dram_tensor(f"gfull{H}", [CORES, 128, BPP, 128], bf16,
                              kind="Internal", addr_space="Shared")
               for H in range(PHASES)]

    with tile.TileContext(nc) as tc, ExitStack() as ctx:
        sb = ctx.enter_context(tc.tile_pool(name="sb", bufs=1))
        lhsp = ctx.enter_context(tc.tile_pool(name="lhsp", bufs=2))
        msgp = ctx.enter_context(tc.tile_pool(name="msgp", bufs=3))
        eidxp = ctx.enter_context(tc.tile_pool(name="eidxp", bufs=3))
        pg = ctx.enter_context(tc.tile_pool(name="pg", bufs=2, space="PSUM"))
        pc = ctx.enter_context(tc.tile_pool(name="pc", bufs=2, space="PSUM"))
        pa = ctx.enter_context(tc.tile_pool(name="pa", bufs=2, space="PSUM"))

        table = sb.tile([128, RP1 + RP2, 128], bf16, tag="table")
        h_fm = sb.tile([128, NPADC], bf16, tag="h_fm")
        dsc = sb.tile([128, PHASES * BPP], f32, tag="dsc")
        nc.sync.dma_start(dsc[:], t_dsc.ap())
        wt = [sb.tile([128, 128], bf16, tag=f"wt{l}", name=f"wt{l}")
              for l in range(2)]
        ct = [sb.tile([128, 4, 128], bf16, tag=f"ct{l}", name=f"ct{l}")
              for l in range(2)]
        bt = [sb.tile([128, 1], f32, tag=f"bt{l}", name=f"bt{l}")
              for l in range(2)]
        for l in range(2):
            nc.sync.dma_start(wt[l][:], t_w[l].ap())
            nc.sync.dma_start(ct[l][:], t_c[l].ap().rearrange("k p f -> p k f"))
            nc.sync.dma_start(bt[l][:], t_b[l].ap())
        wout = sb.tile([128, NCLS], bf16, tag="wout")
        bout = sb.tile([128, NCLS], f32, tag="bout")
        nc.sync.dma_start(wout[:], t_wout.ap())
        nc.sync.dma_start(bout[:], t_bout.ap())
        # ---- L1 table: g1 = W0 @ xs (token-major out). xs staged through
        # the msg pool in 10 big DMAs (keeps the Sync queue short so the
        # first gather's eidx load isn't stuck behind 80 small streams);
        # PSUM drains alternate Scalar/Vector.
        for t in range(NG // 4096):
            xst = msgp.tile([128, 1, MSG_COLS], bf16, tag="msg")
            nc.sync.dma_start(xst[:, 0, :4096],
                              t_xs.ap()[:, t * 4096:(t + 1) * 4096])
            for gg in range(8):
                g = t * 8 + gg
                ps = pg.tile([128, 4, 128], f32, tag="ps_tab")
                for k in range(4):
                    nc.tensor.matmul(
                        ps[:, k, :],
                        lhsT=xst[:, 0, (gg * 4 + k) * 128:
                                 (gg * 4 + k + 1) * 128],
                        rhs=wt[0][:], start=True, stop=True)
                if g % 2 == 0:
                    nc.scalar.activation(table[:, g * 4:(g + 1) * 4, :],
                                         ps[:], AF.Copy)
                else:
                    nc.vector.tensor_scalar_add(
                        table[:, g * 4:(g + 1) * 4, :], ps[:], 0.0)

        for l in range(2):
            for H in range(PHASES):
                csl = slice(H * LPP, (H + 1) * LPP)
                slabs = {}
                for name in ("npad1", "npad2", "mask1", "mask2", "dinvb"):
                    s_t = sb.tile([128, LPP], bf16, tag=f"sl_{name}",
                                  name=f"sl_{name}")
                    nc.sync.dma_start(s_t[:], t_slab[name, H].ap())
                    slabs[name] = s_t

                sum1 = sb.tile([128, LPP], f32, tag="sum1")
                sum2 = sb.tile([128, LPP], f32, tag="sum2")
                mn1 = sb.tile([128, LPP], bf16, tag="mn1")
                mx1 = sb.tile([128, LPP], bf16, tag="mx1")
                mn2 = sb.tile([128, LPP], bf16, tag="mn2")
                mx2 = sb.tile([128, LPP], bf16, tag="mx2")
                gself = sb.tile([128, LPP], bf16, tag="gself")

                # ---- g_self columns for this (core, phase)
                if l == 0:
                    for g in range(LPP // 512):
                        lhs = lhsp.tile([128, 512], bf16, tag="xs_lhs")
                        nc.sync.dma_start(
                            lhs[:],
                            t_xs_self[H].ap()[:, g * 512:(g + 1) * 512])
                        psq = pc.tile([128, 512], f32, tag="ps_cmb")
                        for k in range(4):
                            nc.tensor.matmul(
                                psq[:, k * 128:(k + 1) * 128],
                                lhsT=wt[0][:],
                                rhs=lhs[:, k * 128:(k + 1) * 128],
                                start=True, stop=True)
                        nc.scalar.activation(
                            gself[:, g * 512:(g + 1) * 512], psq[:], AF.Copy)
                else:
                    for g in range(LPP // 512):
                        h1p = lhsp.tile([128, 512], bf16, tag="h1p")
                        nc.vector.tensor_tensor(
                            out=h1p[:],
                            in0=h_fm[:, H * LPP + g * 512:
                                     H * LPP + (g + 1) * 512],
                            in1=slabs["dinvb"][:, g * 512:(g + 1) * 512],
                            op=OP.mult)
                        psq = pc.tile([128, 512], f32, tag="ps_cmb")
                        for k in range(4):
                            nc.tensor.matmul(
                                psq[:, k * 128:(k + 1) * 128],
                                lhsT=wt[1][:],
                                rhs=h1p[:, k * 128:(k + 1) * 128],
                                start=True, stop=True)
                        nc.scalar.activation(
                            gself[:, g * 512:(g + 1) * 512], psq[:], AF.Copy)

                # ---- edge gathers + reduces per part (part1: 128-lane
                # blocks, part2: 64-lane sub-blocks)
                for Pq, S_a, P_a, chunks, stat3, rk0, rkn, W, pb in (
                        (1, S1, P1, chunks1[H], (sum1, mn1, mx1),
                         0, RP1, 128, H * BPP),
                        (2, S2, P2, chunks2[H], (sum2, mn2, mx2),
                         RP1, RP2, 64, 2 * H * BPP)):
                    view = table[:, rk0:rk0 + rkn, :]
                    sm, mn_x, mx_x = stat3
                    npad = slabs[f"npad{Pq}"]
                    for chb in chunks:
                        q0 = int(P_a[chb[0]] - P_a[pb])
                        qn = int(P_a[chb[-1] + 1] - P_a[pb]) - q0
                        msg = msgp.tile([128, 1, MSG_COLS], bf16, tag="msg")
                        eix = eidxp.tile([128, MSG_COLS // 16], i16,
                                         tag="eix")
                        nc.sync.dma_start(
                            eix[:, :qn // 16],
                            t_eidx[Pq, H].ap()[:, q0 // 16:(q0 + qn) // 16])
                        nc.gpsimd.dma_gather(
                            out_ap=msg[:, :, :qn], in_ap=view,
                            idxs_ap=eix[:, :qn // 16],
                            num_idxs=qn, num_idxs_reg=qn, elem_size=D,
                            transpose=True, sbuf_tokens_per_rank=128,
                            sbuf_free_dim_per_rank=D * 2, single_packet=False)
                        for b in chb:
                            s = int(S_a[b])
                            cb = int(P_a[b] - P_a[pb]) - q0
                            v = msg[:, 0, cb:cb + W * s].rearrange(
                                "p (l s) -> p l s", s=s)
                            c0 = b * W - H * LPP
                            dsl = slice(c0, c0 + W)
                            nc.vector.tensor_reduce(
                                out=sm[:, dsl], in_=v, axis=AX, op=OP.add)
                            nc.vector.tensor_reduce(
                                out=mn_x[:, dsl], in_=v, axis=AX, op=OP.min)
                            nc.vector.tensor_reduce(
                                out=mx_x[:, dsl], in_=v, axis=AX, op=OP.max)
                            tmp = lhsp.tile([128, 128], f32, tag="tmp")
                            nc.vector.tensor_tensor(
                                out=tmp[:, :W], in0=v[:, :, 0],
                                in1=npad[:, dsl], op=OP.mult)
                            nc.vector.tensor_tensor(
                                out=sm[:, dsl], in0=sm[:, dsl],
                                in1=tmp[:, :W], op=OP.subtract)

                # ---- merges (slab-wide): masked parts + self
                nc.vector.tensor_tensor(out=sum1[:], in0=sum1[:],
                                        in1=sum2[:], op=OP.add)
                nc.vector.tensor_tensor(out=sum1[:], in0=sum1[:],
                                        in1=gself[:], op=OP.add)
                nc.vector.tensor_tensor(out=mn1[:], in0=mn1[:],
                                        in1=slabs["mask1"][:], op=OP.add)
                nc.vector.tensor_tensor(out=mx1[:], in0=mx1[:],
                                        in1=slabs["mask1"][:],
                                        op=OP.subtract)
                nc.vector.tensor_tensor(out=mn2[:], in0=mn2[:],
                                        in1=slabs["mask2"][:], op=OP.add)
                nc.vector.tensor_tensor(out=mx2[:], in0=mx2[:],
                                        in1=slabs["mask2"][:],
                                        op=OP.subtract)
                nc.vector.tensor_tensor(out=mn1[:], in0=mn1[:],
                                        in1=mn2[:], op=OP.min)
                nc.vector.tensor_tensor(out=mx1[:], in0=mx1[:],
                                        in1=mx2[:], op=OP.max)
                nc.vector.tensor_tensor(out=mn1[:], in0=mn1[:],
                                        in1=gself[:], op=OP.min)
                nc.vector.tensor_tensor(out=mx1[:], in0=mx1[:],
                                        in1=gself[:], op=OP.max)
                # ---- scalings: add = dinv*sum; mean = add*dinv
                add_t = sb.tile([128, LPP], bf16, tag="mx2")
                mean_t = sb.tile([128, LPP], bf16, tag="mn2")
                nc.vector.tensor_tensor(out=add_t[:], in0=sum1[:],
                                        in1=slabs["dinvb"][:],
                                        op=OP.mult)
                nc.vector.tensor_tensor(out=mean_t[:], in0=add_t[:],
                                        in1=slabs["dinvb"][:],
                                        op=OP.mult)
                nc.vector.tensor_tensor(out=mn1[:], in0=mn1[:],
                                        in1=slabs["dinvb"][:],
                                        op=OP.mult)
                nc.vector.tensor_tensor(out=mx1[:], in0=mx1[:],
                                        in1=slabs["dinvb"][:],
                                        op=OP.mult)

                # ---- combine + bias + relu -> h_fm; for l=0 interleave the
                # A-phase (g2 shard = dinv * (h1 @ W1.T)) per 512-group and
                # fire the AllGather(s) as soon as the shard halves are done.
                for g in range(LPP // 512):
                    psc = pc.tile([128, 512], f32, tag="ps_cmb")
                    for k, st in enumerate((mean_t, add_t, mn1, mx1)):
                        nc.tensor.matmul(
                            psc[:], lhsT=ct[l][:, k, :],
                            rhs=st[:, g * 512:(g + 1) * 512],
                            start=(k == 0), stop=(k == 3))
                    nc.scalar.activation(
                        h_fm[:, H * LPP + g * 512:H * LPP + (g + 1) * 512],
                        psc[:], AF.Relu, bias=bt[l][:], scale=1.0)
                    if l == 0:
                        for b in range(4 * g, 4 * (g + 1)):
                            ps2 = pa.tile([128, 128], f32, tag="ps_a")
                            nc.tensor.matmul(
                                ps2[:],
                                lhsT=h_fm[:, H * LPP + b * 128:
                                          H * LPP + (b + 1) * 128],
                                rhs=wt[1][:], start=True, stop=True)
                            gt = lhsp.tile([128, 128], bf16, tag="gt")
                            nc.scalar.activation(
                                gt[:], ps2[:], AF.Copy,
                                scale=dsc[:, H * BPP + b:H * BPP + b + 1])
                            nc.sync.dma_start(t_gsh[H].ap()[:, b, :], gt[:])
                        if g == 4:
                            nc.gpsimd.collective_compute(
                                "AllGather", OP.bypass,
                                replica_groups=[list(range(CORES))],
                                ins=[t_gsh[H].ap()],
                                outs=[t_gfull[H].ap()])

            if l == 0:
                # ---- stream AllGathered g2 into the table.
                # quarter q covers cols q*1280..(q+1)*1280 = 10 ranks/core:
                # q0,q1,q2 -> part1 ranks c*32 + 10q ..; q3 rows 0..255 ->
                # part1 ranks c*32+30..32, rows 256.. -> part2 RP1+c*8..
                for c in range(CORES):
                    nc.sync.dma_start(
                        table[:, c * 32:c * 32 + 20, :], t_gfull[0].ap()[c])
                for c in range(CORES):
                    nc.sync.dma_start(
                        table[:, c * 32 + 20:c * 32 + 32, :],
                        t_gfull[1].ap()[c][:, 0:12, :])
                    nc.sync.dma_start(
                        table[:, RP1 + c * 8:RP1 + (c + 1) * 8, :],
                        t_gfull[1].ap()[c][:, 12:20, :])

        # ---- logits + log_softmax (batched per phase-half; bout is
        # pre-shifted -30 on host so exp is overflow-free, log_softmax is
        # shift-invariant)
        NCH = NPADC // 128
        HCH = NCH // 2
        lg_all = sb.tile([128, NCH, NCLS], f32, tag="lg_all")
        ex_all = sb.tile([128, NCH, NCLS], bf16, tag="ex_all")
        se_all = sb.tile([128, NCH], f32, tag="se_all")
        ls_all = sb.tile([128, NCH], f32, tag="ls_all")
        for Hb in range(2):
            jsl = slice(Hb * HCH, (Hb + 1) * HCH)
            for j in range(Hb * HCH, (Hb + 1) * HCH):
                ps = pg.tile([128, 4, 128], f32, tag="ps_tab")
                nc.tensor.matmul(ps[:, 0, :NCLS],
                                 lhsT=h_fm[:, j * 128:(j + 1) * 128],
                                 rhs=wout[:], start=True, stop=True)
                nc.vector.tensor_tensor(out=lg_all[:, j, :],
                                        in0=ps[:, 0, :NCLS],
                                        in1=bout[:], op=OP.add)
            nc.scalar.activation(ex_all[:, jsl, :], lg_all[:, jsl, :], AF.Exp)
            nc.vector.tensor_reduce(out=se_all[:, jsl],
                                    in_=ex_all[:, jsl, :], axis=AX, op=OP.add)
            nc.scalar.activation(ls_all[:, jsl], se_all[:, jsl], AF.Ln)
            for j in range(Hb * HCH, (Hb + 1) * HCH):
                lg = lhsp.tile([128, NCLS], f32, tag="lg")
                nc.vector.tensor_scalar_sub(lg[:], lg_all[:, j, :],
                                            ls_all[:, j:j + 1])
                nc.sync.dma_start(
                    t_out.ap().rearrange("(a p) n -> p a n", p=128)[:, j, :],
                    lg[:])

    nc.compile()
    return nc


_CACHE = {}


def kernel(x, edge_index, W0, C0, b0, W1, C1, b1, Wout, bout,
           trace=False, _want_results=False):
    x = np.asarray(x, dtype=np.float32)
    per_core, xs_fm, meta = _host_prep(x, edge_index)
    key = (tuple(meta["S1"]), tuple(meta["S2"]))
    if key not in _CACHE:
        _CACHE[key] = _build_program(meta)
    nc = _CACHE[key]

    shared = dict(
        xs_fm=xs_fm,
        W0T=np.ascontiguousarray(np.asarray(W0, np.float32).T).astype(BF16),
        W1T=np.ascontiguousarray(np.asarray(W1, np.float32).T).astype(BF16),
        C0T=np.ascontiguousarray(np.asarray(C0, np.float32).T).reshape(
            4, 128, 128).astype(BF16),
        C1T=np.ascontiguousarray(np.asarray(C1, np.float32).T).reshape(
            4, 128, 128).astype(BF16),
        b0=np.asarray(b0, np.float32).reshape(128, 1),
        b1=np.asarray(b1, np.float32).reshape(128, 1),
        WoutT=np.ascontiguousarray(
            np.asarray(Wout, np.float32).T).astype(BF16),
        boutb=np.broadcast_to(np.asarray(bout, np.float32) - 30.0,
                              (128, NCLS)).copy(),
    )
    in_maps = []
    for c in range(CORES):
        d = per_core[c]
        m = dict(shared)
        for k in ("eidx_p1_0", "eidx_p1_1", "eidx_p2_0", "eidx_p2_1",
                  "npad1_0", "npad1_1", "npad2_0", "npad2_1",
                  "mask1_0", "mask1_1", "mask2_0", "mask2_1",
                  "dinvb_0", "dinvb_1", "dsc"):
            m[k] = d[k]
        # xs_self{H}: feature-major xs columns of this core's phase-H dests,
        # in lane order (cols H*2560..H*2560+2560 -> tokens, part-dependent)
        for H in range(PHASES):
            cols = np.arange(H * LPP, (H + 1) * LPP)
            toks = np.where(cols < P1C, c * P1C + cols,
                            NP1 + c * P2C + (cols - P1C))
            m[f"xs_self{H}"] = np.ascontiguousarray(xs_fm[:, toks])
        in_maps.append(m)

    res = bass_utils.run_bass_kernel_spmd(
        nc, in_maps, core_ids=list(range(CORES)), trace=trace)

    out = np.zeros((N, NCLS), dtype=np.float32)
    for c in range(CORES):
        o = res.results[c]["out"]
        d = per_core[c]
        out[d["gl"][d["real"]]] = o[d["real"]]
    if _want_results:
        return out, res
    return out


# revision 5
# speedup vs baseline: 1.5056x; 1.5056x over previous
"""GCN (2-layer, mean/add/min/max aggregation) Trainium2 Bass kernel, 8 NeuronCores.

v3: eliminates all table-build dma_gather descriptors (Q7 SWDGE descriptor
generation at ~6-8ns/desc is the machine bottleneck for indexed access).

Per core (5000 dest nodes, 2 dest-phases x 2560 padded lanes = 40 blocks):
- Token space NG=40960 split asymmetrically for int16 gather indices:
  part1 = 32768 tokens (cols 0..4095 of each core), part2 = 8192 tokens
  (cols 4096..5119 = each core's 1024 lowest-OUT-degree nodes, so only ~13%
  of edge sources hit part2).
- SBUF-resident full g-table [128, 320, 128] bf16, token-major. Layer 1:
  built by PE matmul from a streamed feature-major xs (= dinv*x) input.
  Layer 2: cores compute g2 = dinv*(h1 @ W1.T) for own nodes (A-phase), two
  phase-half AllGathers move it, table bulk-streamed back with large HWDGE
  descriptors. Zero 256B random table gathers.
- Edge messages: per (layer, phase, part) SBUF->SBUF dma_gather from the part
  view (relative idx < 32768), lane-major uniform slots per 128-dest block,
  tensor_reduce add/min/max + pad correction (pad = slot0 dup). Dests sorted
  by part1-count (~total degree) within col ranges to minimize slot padding.
- The appended self-loop message is NOT gathered: g_self columns (one matmul
  on a contiguous slice) merge into add/min/max afterward. Lanes with no
  sources in a part are neutralized via +-1e30 mask slabs; their partial sum
  self-cancels through the standard pad correction.
"""
import sys

sys.path.insert(0, "/opt/trn_rl_repo")

import numpy as np
import ml_dtypes
from contextlib import ExitStack

import concourse.bacc as bacc
import concourse.tile as tile
import concourse.mybir as mybir
from concourse import bass_utils

N = 40000
E = 640000
D = 128
NCLS = 40
CORES = 8
NPC = N // CORES            # 5000 nodes/core
PHASES = 2
BPP = 20                    # blocks per phase
LPP = BPP * 128             # 2560 lanes/phase
NPADC = PHASES * LPP        # 5120 padded nodes/core
NG = CORES * NPADC          # 40960 tokens
P1C = 4096                  # part1 cols per core
P2C = NPADC - P1C           # 1024 part2 cols per core
NP1 = CORES * P1C           # 32768 part1 tokens
NP2 = CORES * P2C           # 8192
RP1 = NP1 // 128            # 256 part1 ranks
RP2 = NP2 // 128            # 64
MSG_COLS = 4480             # max slots per gather call
BF16 = ml_dtypes.bfloat16
BIG = np.float32(1e30)


def _wrap_idx(idx):
    """int16 -> [128, n/16] wrapped (i -> [i%16, i//16]) and replicated x8."""
    idx = np.asarray(idx, dtype=np.int16)
    n = len(idx)
    assert n % 16 == 0
    cols = n // 16
    base = np.zeros((16, cols), dtype=np.int16)
    base[np.arange(n) % 16, np.arange(n) // 16] = idx
    return np.tile(base, (8, 1))


def _host_prep(x, edge_index):
    row_e = np.asarray(edge_index[0], dtype=np.int64)
    col_e = np.asarray(edge_index[1], dtype=np.int64)
    deg_e = np.bincount(col_e, minlength=N)
    out_e = np.bincount(row_e, minlength=N)
    deg = (deg_e + 1).astype(np.float64)
    dinv = deg ** -0.5

    # ---- part2 selection (per-core 1024 lowest out-degree) + counts
    part2 = np.zeros(N, dtype=bool)
    for c in range(CORES):
        sel = np.argsort(out_e[c * NPC:(c + 1) * NPC], kind="stable")[:P2C]
        part2[c * NPC + sel] = True
    cnt2 = np.bincount(col_e[part2[row_e]], minlength=N)
    cnt1 = deg_e - cnt2

    # ---- column assignment: part1 cols 0..4095 / part2 cols 4096..5119,
    # sorted by (part1 cnt desc, part2 cnt desc) within each range
    col_of = np.zeros(N, dtype=np.int64)
    lanes_of = {}           # c -> [5120] node id or -1
    for c in range(CORES):
        nid0 = c * NPC
        loc2 = nid0 + np.nonzero(part2[nid0:nid0 + NPC])[0]
        loc1 = nid0 + np.nonzero(~part2[nid0:nid0 + NPC])[0]
        p1s = loc1[np.lexsort((-cnt2[loc1], -cnt1[loc1]))]
        p2s = loc2[np.lexsort((-cnt2[loc2], -cnt1[loc2]))]
        lanes = np.full(NPADC, -1, dtype=np.int64)
        lanes[:len(p1s)] = p1s
        lanes[P1C:P1C + len(p2s)] = p2s
        col_of[p1s] = np.arange(len(p1s))
        col_of[p2s] = P1C + np.arange(len(p2s))
        lanes_of[c] = lanes
    core_of = np.arange(N) // NPC
    # token id: part1: c*4096+col (< 32768); part2: 32768 + c*1024 + (col-4096)
    tok_of = np.where(~part2, core_of * P1C + col_of,
                      NP1 + core_of * P2C + (col_of - P1C))

    # ---- uniform slot counts, global over cores.
    # part1: 128-lane blocks; part2: 64-lane sub-blocks (less padding), with
    # per-128-block even total so gather calls stay 128-multiple.
    NB = PHASES * BPP
    S1 = np.ones(NB, dtype=np.int64)
    S2 = np.ones(2 * NB, dtype=np.int64)
    for c in range(CORES):
        lanes = lanes_of[c]
        for b in range(NB):
            ls = lanes[b * 128:(b + 1) * 128]
            ls = ls[ls >= 0]
            if len(ls):
                S1[b] = max(S1[b], cnt1[ls].max())
        for u in range(2 * NB):
            ls = lanes[u * 64:(u + 1) * 64]
            ls = ls[ls >= 0]
            if len(ls):
                S2[u] = max(S2[u], cnt2[ls].max())
    for b in range(NB):
        if (S2[2 * b] + S2[2 * b + 1]) % 2:
            S2[2 * b + 1] += 1
    P1 = np.zeros(NB + 1, dtype=np.int64)
    P2 = np.zeros(2 * NB + 1, dtype=np.int64)
    P1[1:] = np.cumsum(128 * S1)
    P2[1:] = np.cumsum(64 * S2)
    LEN1 = [int(P1[(H + 1) * BPP] - P1[H * BPP]) for H in range(PHASES)]
    LEN2 = [int(P2[2 * (H + 1) * BPP] - P2[2 * H * BPP])
            for H in range(PHASES)]

    def chunk_units(S, H, W, upb):
        # chunk whole 128-blocks (upb units per block) into gather calls
        chunks, cur, cols = [], [], 0
        for b in range(H * BPP, (H + 1) * BPP):
            us = list(range(b * upb, (b + 1) * upb))
            w = sum(W * int(S[u]) for u in us)
            if cur and cols + w > MSG_COLS:
                chunks.append(cur)
                cur, cols = [], 0
            cur.extend(us)
            cols += w
        if cur:
            chunks.append(cur)
        return chunks
    chunks1 = [chunk_units(S1, H, 128, 1) for H in range(PHASES)]
    chunks2 = [chunk_units(S2, H, 64, 2) for H in range(PHASES)]

    # ---- per-dest source lists sorted by (dest, part): part1 prefix
    key = col_e * 2 + part2[row_e]
    eorder = np.argsort(key, kind="stable")
    srcs_tok = tok_of[row_e[eorder]]
    offk = np.zeros(2 * N + 1, dtype=np.int64)
    offk[1:] = np.cumsum(np.bincount(key, minlength=2 * N))

    # ---- per-core arrays
    per_core = []
    for c in range(CORES):
        lanes = lanes_of[c]
        ed1 = [np.zeros(LEN1[H], dtype=np.int64) for H in range(PHASES)]
        ed2 = [np.zeros(LEN2[H], dtype=np.int64) for H in range(PHASES)]
        npad1 = np.zeros(NPADC, dtype=np.float32)
        npad2 = np.zeros(NPADC, dtype=np.float32)
        mask1 = np.zeros(NPADC, dtype=np.float32)
        mask2 = np.zeros(NPADC, dtype=np.float32)
        dinv_l = np.zeros(NPADC, dtype=np.float32)
        dsc = np.zeros((128, NB), dtype=np.float32)
        gl = np.zeros(NPADC, dtype=np.int64)
        real = np.zeros(NPADC, dtype=bool)
        units = [(1, b, b * 128, 128, int(S1[b]),
                  int(P1[b] - P1[(b // BPP) * BPP])) for b in range(NB)]
        units += [(2, u, u * 64, 64, int(S2[u]),
                   int(P2[u] - P2[2 * (u // (2 * BPP)) * BPP]))
                  for u in range(2 * NB)]
        for P, b, col0, W, S, Poff in units:
            H = col0 // LPP
            ls = lanes[col0:col0 + W]
            if True:
                ed, npad, mask, base_rel = (
                    (ed1[H], npad1, mask1, 0) if P == 1 else
                    (ed2[H], npad2, mask2, NP1))
                M = np.zeros((W, S), dtype=np.int64)
                for l in range(W):
                    col = col0 + l
                    d = ls[l]
                    if d < 0:
                        # pad lane: own (part1) pad token exists at this col
                        if P == 1:
                            M[l, :] = c * P1C + col
                        else:
                            M[l, :] = 0
                            mask[col] = BIG
                        npad[col] = S
                        continue
                    o0 = offk[2 * d + (P - 1)]
                    o1 = offk[2 * d + P]
                    k = o1 - o0
                    if k:
                        t = srcs_tok[o0:o1] - base_rel
                        M[l, :k] = t
                        M[l, k:] = t[0]
                    else:
                        selfp2 = part2[d]
                        if (P == 2) == selfp2:
                            M[l, :] = tok_of[d] - base_rel   # self fallback
                        else:
                            M[l, :] = 0
                            mask[col] = BIG
                    npad[col] = S - k
                ed[Poff:Poff + W * S] = M.reshape(-1)
        for b in range(NB):
            ls = lanes[b * 128:(b + 1) * 128]
            for l in range(128):
                col = b * 128 + l
                d = ls[l]
                if d >= 0:
                    dinv_l[col] = dinv[d]
                    dsc[l, b] = dinv[d]
                    gl[col] = d
                    real[col] = True

        def slab(a, H):
            return np.broadcast_to(a[H * LPP:(H + 1) * LPP],
                                   (128, LPP)).astype(BF16).copy()
        pcd = dict(
            eidx_p1_0=_wrap_idx(ed1[0]), eidx_p1_1=_wrap_idx(ed1[1]),
            eidx_p2_0=_wrap_idx(ed2[0]), eidx_p2_1=_wrap_idx(ed2[1]),
            dsc=dsc, gl=gl, real=real,
        )
        for H in range(PHASES):
            pcd[f"npad1_{H}"] = slab(npad1, H)
            pcd[f"npad2_{H}"] = slab(npad2, H)
            pcd[f"mask1_{H}"] = slab(mask1, H)
            pcd[f"mask2_{H}"] = slab(mask2, H)
            pcd[f"dinvb_{H}"] = slab(dinv_l, H)
        per_core.append(pcd)

    # ---- xs feature-major, token-ordered, dinv-prescaled
    xs = dinv[:, None] * np.asarray(x, dtype=np.float64)
    xs_fm = np.zeros((D, NG), dtype=np.float32)
    xs_fm[:, tok_of] = xs.T.astype(np.float32)

    meta = dict(S1=S1, S2=S2, P1=P1, P2=P2, LEN1=LEN1, LEN2=LEN2,
                chunks1=chunks1, chunks2=chunks2)
    return per_core, xs_fm.astype(BF16), meta


def _build_program(meta):
    S1, S2, P1, P2 = meta["S1"], meta["S2"], meta["P1"], meta["P2"]
    LEN1, LEN2 = meta["LEN1"], meta["LEN2"]
    chunks1, chunks2 = meta["chunks1"], meta["chunks2"]
    f32, bf16, i16 = mybir.dt.float32, mybir.dt.bfloat16, mybir.dt.int16
    AX = mybir.AxisListType.X
    OP = mybir.AluOpType
    AF = mybir.ActivationFunctionType

    nc = bacc.Bacc("TRN2", target_bir_lowering=False, debug=False,
                   num_devices=CORES, num_swdge_queues=2)
    t_xs = nc.dram_tensor("xs_fm", [128, NG], bf16, kind="ExternalInput")
    t_w = [nc.dram_tensor(f"W{l}T", [128, 128], bf16, kind="ExternalInput")
           for l in range(2)]
    t_c = [nc.dram_tensor(f"C{l}T", [4, 128, 128], bf16, kind="ExternalInput")
           for l in range(2)]
    t_b = [nc.dram_tensor(f"b{l}", [128, 1], f32, kind="ExternalInput")
           for l in range(2)]
    t_wout = nc.dram_tensor("WoutT", [128, NCLS], bf16, kind="ExternalInput")
    t_bout = nc.dram_tensor("boutb", [128, NCLS], f32, kind="ExternalInput")
    t_dsc = nc.dram_tensor("dsc", [128, PHASES * BPP], f32,
                           kind="ExternalInput")
    t_xs_self = [nc.dram_tensor(f"xs_self{H}", [128, LPP], bf16,
                                kind="ExternalInput") for H in range(PHASES)]
    t_eidx = {}
    for H in range(PHASES):
        t_eidx[1, H] = nc.dram_tensor(
            f"eidx_p1_{H}", [128, LEN1[H] // 16], i16, kind="ExternalInput")
        t_eidx[2, H] = nc.dram_tensor(
            f"eidx_p2_{H}", [128, LEN2[H] // 16], i16, kind="ExternalInput")
    t_slab = {}
    for name in ("npad1", "npad2", "mask1", "mask2", "dinvb"):
        for H in range(PHASES):
            t_slab[name, H] = nc.dram_tensor(
                f"{name}_{H}", [128, LPP], bf16, kind="ExternalInput")
    t_out = nc.dram_tensor("out", [NPADC, NCLS], f32, kind="ExternalOutput")
    t_gsh = [nc.dram_tensor(f"gsh{H}", [128, BPP, 128], bf16, kind="Internal")
             for H in range(PHASES)]
    t_gfull = [nc.dram_tensor(f"gfull{H}", [CORES, 128, BPP, 128], bf16,
                              kind="Internal", addr_space="Shared")
               for H in range(PHASES)]

    with tile.TileContext(nc) as tc, ExitStack() as ctx:
        sb = ctx.enter_context(tc.tile_pool(name="sb", bufs=1))
        lhsp = ctx.enter_context(tc.tile_pool(name="lhsp", bufs=2))
        msgp = ctx.enter_context(tc.tile_pool(name="msgp", bufs=3))
        eidxp = ctx.enter_context(tc.tile_pool(name="eidxp", bufs=3))
        pg = ctx.enter_context(tc.tile_pool(name="pg", bufs=2, space="PSUM"))
        pc = ctx.enter_context(tc.tile_pool(name="pc", bufs=2, space="PSUM"))
        pa = ctx.enter_context(tc.tile_pool(name="pa", bufs=2, space="PSUM"))

        table = sb.tile([128, RP1 + RP2, 128], bf16, tag="table")
        h_fm = sb.tile([128, NPADC], bf16, tag="h_fm")
        dsc = sb.tile([128, PHASES * BPP], f32, tag="dsc")
        nc.sync.dma_start(dsc[:], t_dsc.ap())
        wt = [sb.tile([128, 128], bf16, tag=f"wt{l}", name=f"wt{l}")
              for l in range(2)]
        ct = [sb.tile([128, 4, 128], bf16, tag=f"ct{l}", name=f"ct{l}")
              for l in range(2)]
        bt = [sb.tile([128, 1], f32, tag=f"bt{l}", name=f"bt{l}")
              for l in range(2)]
        for l in range(2):
            nc.sync.dma_start(wt[l][:], t_w[l].ap())
            nc.sync.dma_start(ct[l][:], t_c[l].ap().rearrange("k p f -> p k f"))
            nc.sync.dma_start(bt[l][:], t_b[l].ap())
        wout = sb.tile([128, NCLS], bf16, tag="wout")
        bout = sb.tile([128, NCLS], f32, tag="bout")
        nc.sync.dma_start(wout[:], t_wout.ap())
        nc.sync.dma_start(bout[:], t_bout.ap())
        # ---- L1 table: g1 = W0 @ xs (token-major out). xs staged through
        # the msg pool in 10 big DMAs (keeps the Sync queue short so the
        # first gather's eidx load isn't stuck behind 80 small streams);
        # PSUM drains alternate Scalar/Vector.
        for t in range(NG // 4096):
            xst = msgp.tile([128, 1, MSG_COLS], bf16, tag="msg")
            nc.sync.dma_start(xst[:, 0, :4096],
                              t_xs.ap()[:, t * 4096:(t + 1) * 4096])
            for gg in range(8):
                g = t * 8 + gg
                ps = pg.tile([128, 4, 128], f32, tag="ps_tab")
                for k in range(4):
                    nc.tensor.matmul(
                        ps[:, k, :],
                        lhsT=xst[:, 0, (gg * 4 + k) * 128:
                                 (gg * 4 + k + 1) * 128],
                        rhs=wt[0][:], start=True, stop=True)
                if g % 2 == 0:
                    nc.scalar.activation(table[:, g * 4:(g + 1) * 4, :],
                                         ps[:], AF.Copy)
                else:
                    nc.vector.tensor_scalar_add(
                        table[:, g * 4:(g + 1) * 4, :], ps[:], 0.0)

        gq = [0]
        for l in range(2):
            for H in range(PHASES):
                csl = slice(H * LPP, (H + 1) * LPP)
                slabs = {}
                for name in ("npad1", "npad2", "mask1", "mask2", "dinvb"):
                    s_t = sb.tile([128, LPP], bf16, tag=f"sl_{name}",
                                  name=f"sl_{name}")
                    nc.sync.dma_start(s_t[:], t_slab[name, H].ap())
                    slabs[name] = s_t

                sum1 = sb.tile([128, LPP], f32, tag="sum1")
                sum2 = sb.tile([128, LPP], f32, tag="sum2")
                mn1 = sb.tile([128, LPP], bf16, tag="mn1")
                mx1 = sb.tile([128, LPP], bf16, tag="mx1")
                mn2 = sb.tile([128, LPP], bf16, tag="mn2")
                mx2 = sb.tile([128, LPP], bf16, tag="mx2")
                gself = sb.tile([128, LPP], bf16, tag="gself")

                # ---- g_self columns for this (core, phase)
                if l == 0:
                    for g in range(LPP // 512):
                        lhs = lhsp.tile([128, 512], bf16, tag="xs_lhs")
                        nc.sync.dma_start(
                            lhs[:],
                            t_xs_self[H].ap()[:, g * 512:(g + 1) * 512])
                        psq = pc.tile([128, 512], f32, tag="ps_cmb")
                        for k in range(4):
                            nc.tensor.matmul(
                                psq[:, k * 128:(k + 1) * 128],
                                lhsT=wt[0][:],
                                rhs=lhs[:, k * 128:(k + 1) * 128],
                                start=True, stop=True)
                        nc.scalar.activation(
                            gself[:, g * 512:(g + 1) * 512], psq[:], AF.Copy)
                else:
                    for g in range(LPP // 512):
                        h1p = lhsp.tile([128, 512], bf16, tag="h1p")
                        nc.vector.tensor_tensor(
                            out=h1p[:],
                            in0=h_fm[:, H * LPP + g * 512:
                                     H * LPP + (g + 1) * 512],
                            in1=slabs["dinvb"][:, g * 512:(g + 1) * 512],
                            op=OP.mult)
                        psq = pc.tile([128, 512], f32, tag="ps_cmb")
                        for k in range(4):
                            nc.tensor.matmul(
                                psq[:, k * 128:(k + 1) * 128],
                                lhsT=wt[1][:],
                                rhs=h1p[:, k * 128:(k + 1) * 128],
                                start=True, stop=True)
                        nc.scalar.activation(
                            gself[:, g * 512:(g + 1) * 512], psq[:], AF.Copy)

                # ---- edge gathers + reduces per part (part1: 128-lane
                # blocks, part2: 64-lane sub-blocks)
                for Pq, S_a, P_a, chunks, stat3, rk0, rkn, W, pb in (
                        (1, S1, P1, chunks1[H], (sum1, mn1, mx1),
                         0, RP1, 128, H * BPP),
                        (2, S2, P2, chunks2[H], (sum2, mn2, mx2),
                         RP1, RP2, 64, 2 * H * BPP)):
                    view = table[:, rk0:rk0 + rkn, :]
                    sm, mn_x, mx_x = stat3
                    npad = slabs[f"npad{Pq}"]
                    for chb in chunks:
                        q0 = int(P_a[chb[0]] - P_a[pb])
                        qn = int(P_a[chb[-1] + 1] - P_a[pb]) - q0
                        msg = msgp.tile([128, 1, MSG_COLS], bf16, tag="msg")
                        eix = eidxp.tile([128, MSG_COLS // 16], i16,
                                         tag="eix")
                        nc.sync.dma_start(
                            eix[:, :qn // 16],
                            t_eidx[Pq, H].ap()[:, q0 // 16:(q0 + qn) // 16])
                        gq[0] ^= 1
                        nc.gpsimd.dma_gather(
                            out_ap=msg[:, :, :qn], in_ap=view,
                            idxs_ap=eix[:, :qn // 16],
                            num_idxs=qn, num_idxs_reg=qn, elem_size=D,
                            transpose=True, sbuf_tokens_per_rank=128,
                            sbuf_free_dim_per_rank=D * 2, single_packet=False,
                            queue_num=gq[0])
                        for b in chb:
                            s = int(S_a[b])
                            cb = int(P_a[b] - P_a[pb]) - q0
                            v = msg[:, 0, cb:cb + W * s].rearrange(
                                "p (l s) -> p l s", s=s)
                            c0 = b * W - H * LPP
                            dsl = slice(c0, c0 + W)
                            nc.vector.tensor_reduce(
                                out=sm[:, dsl], in_=v, axis=AX, op=OP.add)
                            nc.vector.tensor_reduce(
                                out=mn_x[:, dsl], in_=v, axis=AX, op=OP.min)
                            nc.vector.tensor_reduce(
                                out=mx_x[:, dsl], in_=v, axis=AX, op=OP.max)
                            tmp = lhsp.tile([128, 128], f32, tag="tmp")
                            nc.vector.tensor_tensor(
                                out=tmp[:, :W], in0=v[:, :, 0],
                                in1=npad[:, dsl], op=OP.mult)
                            nc.vector.tensor_tensor(
                                out=sm[:, dsl], in0=sm[:, dsl],
                                in1=tmp[:, :W], op=OP.subtract)

                # ---- merges (slab-wide): masked parts + self
                nc.vector.tensor_tensor(out=sum1[:], in0=sum1[:],
                                        in1=sum2[:], op=OP.add)
                nc.vector.tensor_tensor(out=sum1[:], in0=sum1[:],
                                        in1=gself[:], op=OP.add)
                nc.vector.tensor_tensor(out=mn1[:], in0=mn1[:],
                                        in1=slabs["mask1"][:], op=OP.add)
                nc.vector.tensor_tensor(out=mx1[:], in0=mx1[:],
                                        in1=slabs["mask1"][:],
                                        op=OP.subtract)
                nc.vector.tensor_tensor(out=mn2[:], in0=mn2[:],
                                        in1=slabs["mask2"][:], op=OP.add)
                nc.vector.tensor_tensor(out=mx2[:], in0=mx2[:],
                                        in1=slabs["mask2"][:],
                                        op=OP.subtract)
                nc.vector.tensor_tensor(out=mn1[:], in0=mn1[:],
                                        in1=mn2[:], op=OP.min)
                nc.vector.tensor_tensor(out=mx1[:], in0=mx1[:],
                                        in1=mx2[:], op=OP.max)
                nc.vector.tensor_tensor(out=mn1[:], in0=mn1[:],
                                        in1=gself[:], op=OP.min)
                nc.vector.tensor_tensor(out=mx1[:], in0=mx1[:],
                                        in1=gself[:], op=OP.max)
                # ---- scalings: add = dinv*sum; mean = add*dinv
                add_t = sb.tile([128, LPP], bf16, tag="mx2")
                mean_t = sb.tile([128, LPP], bf16, tag="mn2")
                nc.vector.tensor_tensor(out=add_t[:], in0=sum1[:],
                                        in1=slabs["dinvb"][:],
                                        op=OP.mult)
                nc.vector.tensor_tensor(out=mean_t[:], in0=add_t[:],
                                        in1=slabs["dinvb"][:],
                                        op=OP.mult)
                nc.vector.tensor_tensor(out=mn1[:], in0=mn1[:],
                                        in1=slabs["dinvb"][:],
                                        op=OP.mult)
                nc.vector.tensor_tensor(out=mx1[:], in0=mx1[:],
                                        in1=slabs["dinvb"][:],
                                        op=OP.mult)

                # ---- combine + bias + relu -> h_fm; for l=0 interleave the
                # A-phase (g2 shard = dinv * (h1 @ W1.T)) per 512-group and
                # fire the AllGather(s) as soon as the shard halves are done.
                for g in range(LPP // 512):
                    psc = pc.tile([128, 512], f32, tag="ps_cmb")
                    for k, st in enumerate((mean_t, add_t, mn1, mx1)):
                        nc.tensor.matmul(
                            psc[:], lhsT=ct[l][:, k, :],
                            rhs=st[:, g * 512:(g + 1) * 512],
                            start=(k == 0), stop=(k == 3))
                    nc.scalar.activation(
                        h_fm[:, H * LPP + g * 512:H * LPP + (g + 1) * 512],
                        psc[:], AF.Relu, bias=bt[l][:], scale=1.0)
                    if l == 0:
                        for b in range(4 * g, 4 * (g + 1)):
                            ps2 = pa.tile([128, 128], f32, tag="ps_a")
                            nc.tensor.matmul(
                                ps2[:],
                                lhsT=h_fm[:, H * LPP + b * 128:
                                          H * LPP + (b + 1) * 128],
                                rhs=wt[1][:], start=True, stop=True)
                            gt = lhsp.tile([128, 128], bf16, tag="gt")
                            nc.scalar.activation(
                                gt[:], ps2[:], AF.Copy,
                                scale=dsc[:, H * BPP + b:H * BPP + b + 1])
                            nc.sync.dma_start(t_gsh[H].ap()[:, b, :], gt[:])
                        if g == 4:
                            nc.gpsimd.collective_compute(
                                "AllGather", OP.bypass,
                                replica_groups=[list(range(CORES))],
                                ins=[t_gsh[H].ap()],
                                outs=[t_gfull[H].ap()])

            if l == 0:
                # ---- stream AllGathered g2 into the table.
                # quarter q covers cols q*1280..(q+1)*1280 = 10 ranks/core:
                # q0,q1,q2 -> part1 ranks c*32 + 10q ..; q3 rows 0..255 ->
                # part1 ranks c*32+30..32, rows 256.. -> part2 RP1+c*8..
                for c in range(CORES):
                    nc.sync.dma_start(
                        table[:, c * 32:c * 32 + 20, :], t_gfull[0].ap()[c])
                for c in range(CORES):
                    nc.sync.dma_start(
                        table[:, c * 32 + 20:c * 32 + 32, :],
                        t_gfull[1].ap()[c][:, 0:12, :])
                    nc.sync.dma_start(
                        table[:, RP1 + c * 8:RP1 + (c + 1) * 8, :],
                        t_gfull[1].ap()[c][:, 12:20, :])

        # ---- logits + log_softmax (batched per phase-half; bout is
        # pre-shifted -30 on host so exp is overflow-free, log_softmax is
        # shift-invariant)
        NCH = NPADC // 128
        HCH = NCH // 2
        lg_all = sb.tile([128, NCH, NCLS], f32, tag="lg_all")
        ex_all = sb.tile([128, NCH, NCLS], bf16, tag="ex_all")
        se_all = sb.tile([128, NCH], f32, tag="se_all")
        ls_all = sb.tile([128, NCH], f32, tag="ls_all")
        for Hb in range(2):
            jsl = slice(Hb * HCH, (Hb + 1) * HCH)
            for j in range(Hb * HCH, (Hb + 1) * HCH):
                ps = pg.tile([128, 4, 128], f32, tag="ps_tab")
                nc.tensor.matmul(ps[:, 0, :NCLS],
                                 lhsT=h_fm[:, j * 128:(j + 1) * 128],
                                 rhs=wout[:], start=True, stop=True)
                nc.vector.tensor_tensor(out=lg_all[:, j, :],
                                        in0=ps[:, 0, :NCLS],
                                        in1=bout[:], op=OP.add)
            nc.scalar.activation(ex_all[:, jsl, :], lg_all[:, jsl, :], AF.Exp)
            nc.vector.tensor_reduce(out=se_all[:, jsl],
                                    in_=ex_all[:, jsl, :], axis=AX, op=OP.add)
            nc.scalar.activation(ls_all[:, jsl], se_all[:, jsl], AF.Ln)
            for j in range(Hb * HCH, (Hb + 1) * HCH):
                lg = lhsp.tile([128, NCLS], f32, tag="lg")
                nc.vector.tensor_scalar_sub(lg[:], lg_all[:, j, :],
                                            ls_all[:, j:j + 1])
                nc.sync.dma_start(
                    t_out.ap().rearrange("(a p) n -> p a n", p=128)[:, j, :],
                    lg[:])

    nc.compile()
    return nc


_CACHE = {}


def kernel(x, edge_index, W0, C0, b0, W1, C1, b1, Wout, bout,
           trace=False, _want_results=False):
    x = np.asarray(x, dtype=np.float32)
    per_core, xs_fm, meta = _host_prep(x, edge_index)
    key = (tuple(meta["S1"]), tuple(meta["S2"]))
    if key not in _CACHE:
        _CACHE[key] = _build_program(meta)
    nc = _CACHE[key]

    shared = dict(
        xs_fm=xs_fm,
        W0T=np.ascontiguousarray(np.asarray(W0, np.float32).T).astype(BF16),
        W1T=np.ascontiguousarray(np.asarray(W1, np.float32).T).astype(BF16),
        C0T=np.ascontiguousarray(np.asarray(C0, np.float32).T).reshape(
            4, 128, 128).astype(BF16),
        C1T=np.ascontiguousarray(np.asarray(C1, np.float32).T).reshape(
            4, 128, 128).astype(BF16),
        b0=np.asarray(b0, np.float32).reshape(128, 1),
        b1=np.asarray(b1, np.float32).reshape(128, 1),
        WoutT=np.ascontiguousarray(
            np.asarray(Wout, np.float32).T).astype(BF16),
        boutb=np.broadcast_to(np.asarray(bout, np.float32) - 30.0,
                              (128, NCLS)).copy(),
    )
    in_maps = []
    for c in range(CORES):
        d = per_core[c]
        m = dict(shared)
        for k in ("eidx_p1_0", "eidx_p1_1", "eidx_p2_0", "eidx_p2_1",
                  "npad1_0", "npad1_1", "npad2_0", "npad2_1",
                  "mask1_0", "mask1_1", "mask2_0", "mask2_1",
                  "dinvb_0", "dinvb_1", "dsc"):
            m[k] = d[k]
        # xs_self{H}: feature-major xs columns of this core's phase-H dests,
        # in lane order (cols H*2560..H*2560+2560 -> tokens, part-dependent)
        for H in range(PHASES):
            cols = np.arange(H * LPP, (H + 1) * LPP)
            toks = np.where(cols < P1C, c * P1C + cols,
                            NP1 + c * P2C + (cols - P1C))
            m[f"xs_self{H}"] = np.ascontiguousarray(xs_fm[:, toks])
        in_maps.append(m)

    res = bass_utils.run_bass_kernel_spmd(
        nc, in_maps, core_ids=list(range(CORES)), trace=trace)

    out = np.zeros((N, NCLS), dtype=np.float32)
    for c in range(CORES):
        o = res.results[c]["out"]
        d = per_core[c]
        out[d["gl"][d["real"]]] = o[d["real"]]
    if _want_results:
        return out, res
    return out
